# revision 12
# baseline (speedup 1.0000x reference)
"""MoLoRA Trainium2 Bass kernel — r8: paired DMA variant of r6.

Same compute tiling as r6 (TT=256 per tile, f32r matmuls, copies + fp32
SWDGE accum + lagged store), but the three HBM streams move in 1.28 MB
transfers covering TWO compute tiles each: half the DMA count, better
per-transfer efficiency, half the semaphore traffic on the DMA queues.
Pipeline lags are in pair units: delta lags 1 pair, accum 2, store 3.
"""

import numpy as np
from contextlib import ExitStack

import concourse.bass as bass
import concourse.tile as tile
from concourse import bacc
from concourse import mybir
from concourse.bass import ts
from concourse.masks import make_identity
from concourse.bass_utils import run_bass_kernel_spmd

F32 = mybir.dt.float32
F32R = mybir.dt.float32r
AF = mybir.ActivationFunctionType
ALU = mybir.AluOpType
AX = mybir.AxisListType

H = 640
E = 5
R = 8
ER = E * R
RH = 256
HC = H // 128
RC = RH // 128
SCALING = 16.0 / R
N_CORES = 8
T_CORE = 4096
TT = 256          # compute tile (2 halves of 128 tokens)
PT = 2 * TT       # DMA pair tile (512 tokens, 1.28 MB per stream)


def build_kernel(t_core=T_CORE, niter=1, timing_mode=False, passes=1):
    assert t_core % PT == 0
    npairs = t_core // PT
    nc = bacc.Bacc()

    if timing_mode:
        x_d = nc.dram_tensor("x_int", [t_core, H], F32)[:, :]
        base_d = nc.dram_tensor("base_int", [t_core, H], F32)[:, :]
        out_d = nc.dram_tensor("out_int", [t_core, H], F32)[:, :]
        dummy_d = nc.declare_dram_parameter("dummy_out", [1, 4], F32, isOutput=True)
    else:
        x_d = nc.declare_dram_parameter("x", [t_core, H], F32, isOutput=False)
        base_d = nc.declare_dram_parameter("base", [t_core, H], F32, isOutput=False)
        out_d = nc.declare_dram_parameter("out", [t_core, H], F32, isOutput=True)
        dummy_d = None
    w1_d = nc.declare_dram_parameter("W1", [H, RH], F32, isOutput=False)
    b1_d = nc.declare_dram_parameter("b1", [RH], F32, isOutput=False)
    w2_d = nc.declare_dram_parameter("W2", [RH, E], F32, isOutput=False)
    b2_d = nc.declare_dram_parameter("b2", [E], F32, isOutput=False)
    a_d = nc.declare_dram_parameter("A", [E, H, R], F32, isOutput=False)
    bm_d = nc.declare_dram_parameter("Bm", [E, R, H], F32, isOutput=False)

    with ExitStack() as ctx:
        tc = ctx.enter_context(tile.TileContext(nc))
        const = ctx.enter_context(tc.tile_pool(name="const", bufs=1))
        xin_p = ctx.enter_context(tc.tile_pool(name="xin", bufs=3))
        bout_p = ctx.enter_context(tc.tile_pool(name="bout", bufs=4))
        xt_p = ctx.enter_context(tc.tile_pool(name="xt", bufs=3))
        ht_p = ctx.enter_context(tc.tile_pool(name="ht", bufs=2))
        small_p = ctx.enter_context(tc.tile_pool(name="small", bufs=4))
        lw_p = ctx.enter_context(tc.tile_pool(name="lw", bufs=4))
        ps_xt = ctx.enter_context(tc.tile_pool(name="ps_xt", bufs=1, space="PSUM"))
        ps_rt = ctx.enter_context(tc.tile_pool(name="ps_rt", bufs=1, space="PSUM"))
        ps_low = ctx.enter_context(tc.tile_pool(name="ps_low", bufs=2, space="PSUM"))
        ps_wrt = ctx.enter_context(tc.tile_pool(name="ps_wrt", bufs=1, space="PSUM"))
        ps_dl = ctx.enter_context(tc.tile_pool(name="ps_dl", bufs=1, space="PSUM"))

        ident = const.tile([128, 128], F32)
        make_identity(nc, ident)
        ident_r = const.tile([128, 128], F32R)
        nc.vector.tensor_copy(out=ident_r, in_=ident)

        w1_sb = const.tile([128, HC, RH], F32R)
        nc.gpsimd.dma_start(
            out=w1_sb, in_=w1_d.bitcast(F32R).rearrange("(c p) m -> p c m", p=128)
        )
        b1_sb = const.tile([128, RC], F32)
        nc.gpsimd.dma_start(out=b1_sb, in_=b1_d.rearrange("(c p) -> p c", p=128))
        w2_sb = const.tile([128, RC, E], F32)
        nc.gpsimd.dma_start(out=w2_sb, in_=w2_d.rearrange("(c p) e -> p c e", p=128))
        b2_sb = const.tile([1, E], F32)
        nc.gpsimd.dma_start(out=b2_sb, in_=b2_d[:].unsqueeze(0))
        ones_sb = const.tile([1, 128], F32)
        nc.vector.memset(ones_sb, 1.0)
        acat_sb = const.tile([128, HC, E, R], F32R)
        for e in range(E):
            for c in range(HC):
                nc.gpsimd.dma_start(
                    out=acat_sb[:, c, e, :],
                    in_=a_d.bitcast(F32R)[e, c * 128 : (c + 1) * 128, :],
                )
        bcat_sb = const.tile([ER, H], F32R)
        for e in range(E):
            nc.gpsimd.dma_start(
                out=bcat_sb[e * R : (e + 1) * R, :], in_=bm_d.bitcast(F32R)[e, :, :]
            )

        if dummy_d is not None:
            dnm = const.tile([1, 4], F32)
            nc.vector.memset(dnm, 1.0)
            nc.sync.dma_start(out=dummy_d[:, :], in_=dnm)

        loop_ctx = tc.For_i(0, niter, 1) if niter > 1 else None
        if loop_ctx is not None:
            ctx.enter_context(loop_ctx)

        def emit_pair_load(pi):
            """one 1.28 MB x load covering both compute tiles of the pair"""
            tok = pi * PT
            x_nat = xin_p.tile([128, 4, H], F32R)
            nc.sync.dma_start(
                out=x_nat,
                in_=x_d.bitcast(F32R)[tok : tok + PT, :].rearrange(
                    "(q p) h -> p q h", p=128
                ),
            )
            return {"x_nat": x_nat, "tok": tok}

        def emit_front(pair, half):
            """xT transposes for one compute tile of the pair"""
            x_nat = pair["x_nat"]
            xt_sb = xt_p.tile([128, HC, TT], F32R)
            for j in range(2):
                q = 2 * half + j
                xtp = ps_xt.tile([128, HC, 128], F32R, tag="xtp")
                for c in range(HC):
                    nc.tensor.transpose(
                        out=xtp[:, c, :],
                        in_=x_nat[:, q, ts(c, 128)],
                        identity=ident_r,
                    )
                nc.any.tensor_copy(out=xt_sb[:, :, ts(j, 128)], in_=xtp)
            return {"xt_sb": xt_sb}

        def emit_router(st):
            xt_r = st["xt_sb"]
            h_ps = ps_rt.tile([128, RC, TT], F32, tag="rt")
            for c2 in range(RC):
                for c in range(HC):
                    nc.tensor.matmul(
                        out=h_ps[:, c2, :],
                        lhsT=w1_sb[:, c, ts(c2, 128)],
                        rhs=xt_r[:, c, :],
                        start=(c == 0),
                        stop=(c == HC - 1),
                    )
            ht_sb = ht_p.tile([128, RC, TT], F32)
            sg_sb = ht_p.tile([128, RC, TT], F32, tag="sg")
            for c2 in range(RC):
                nc.scalar.activation(
                    out=sg_sb[:, c2, :],
                    in_=h_ps[:, c2, :],
                    func=AF.Sigmoid,
                    bias=b1_sb[:, c2 : c2 + 1],
                )
                nc.vector.scalar_tensor_tensor(
                    out=ht_sb[:, c2, :],
                    in0=h_ps[:, c2, :],
                    scalar=b1_sb[:, c2 : c2 + 1],
                    in1=sg_sb[:, c2, :],
                    op0=ALU.add,
                    op1=ALU.mult,
                )

            low_ps = ps_low.tile([ER, TT], F32, tag="low")
            for c in range(HC):
                nc.tensor.matmul(
                    out=low_ps,
                    lhsT=acat_sb[:, c, :, :],
                    rhs=xt_r[:, c, :],
                    start=(c == 0),
                    stop=(c == HC - 1),
                )
            st["low_ps"] = low_ps

            lg_full = ps_rt.tile([128, RC, TT], F32, tag="rt")
            lg_ps = lg_full[:, :, 0:E]
            for j in range(2):
                for c2 in range(RC):
                    nc.tensor.matmul(
                        out=lg_ps[:, j, :],
                        lhsT=ht_sb[:, c2, ts(j, 128)],
                        rhs=w2_sb[:, c2, :],
                        start=(c2 == 0),
                        stop=False,
                    )
                nc.tensor.matmul(
                    out=lg_ps[:, j, :],
                    lhsT=ones_sb,
                    rhs=b2_sb,
                    start=False,
                    stop=True,
                )

            m1 = small_p.tile([128, 2], F32, tag="m1")
            nc.vector.tensor_reduce(out=m1, in_=lg_ps, axis=AX.X, op=ALU.max)
            top1 = small_p.tile([128, 2, E], F32, tag="top1")
            nc.vector.tensor_tensor(
                out=top1,
                in0=lg_ps,
                in1=m1.unsqueeze(-1).broadcast_to([128, 2, E]),
                op=ALU.is_equal,
            )
            masked = small_p.tile([128, 2, E], F32, tag="masked")
            nc.vector.scalar_tensor_tensor(
                out=masked, in0=top1, scalar=-1e30, in1=lg_ps,
                op0=ALU.mult, op1=ALU.add,
            )
            m2 = small_p.tile([128, 2], F32, tag="m2")
            nc.vector.tensor_reduce(out=m2, in_=masked, axis=AX.X, op=ALU.max)
            dlg = small_p.tile([128, 2], F32, tag="dlg")
            nc.vector.tensor_tensor(out=dlg, in0=m2, in1=m1, op=ALU.subtract)
            st["top1"], st["masked"], st["m2"], st["dlg"] = top1, masked, m2, dlg

        def emit_weights(st):
            top1, masked, m2, dlg = st["top1"], st["masked"], st["m2"], st["dlg"]
            u2 = small_p.tile([128, 2], F32, tag="u2")
            nc.scalar.activation(out=u2, in_=dlg, func=AF.Sigmoid)
            u1 = small_p.tile([128, 2], F32, tag="u1")
            nc.vector.tensor_scalar(
                out=u1, in0=u2, scalar1=-1.0, scalar2=1.0,
                op0=ALU.mult, op1=ALU.add,
            )
            top2 = small_p.tile([128, 2, E], F32, tag="top2")
            nc.vector.tensor_tensor(
                out=top2,
                in0=masked,
                in1=m2.unsqueeze(-1).broadcast_to([128, 2, E]),
                op=ALU.is_equal,
            )
            w_full = small_p.tile([128, 2, ER], F32R)
            wt2 = small_p.tile([128, 2, ER], F32, tag="wt2")
            w4 = w_full.rearrange("p j (e r) -> p j e r", r=R)
            wt24 = wt2.rearrange("p j (e r) -> p j e r", r=R)
            t14 = top1.unsqueeze(-1).broadcast_to([128, 2, E, R])
            t24 = top2.unsqueeze(-1).broadcast_to([128, 2, E, R])
            u14 = u1.unsqueeze(-1).unsqueeze(-1).broadcast_to([128, 2, E, R])
            u24 = u2.unsqueeze(-1).unsqueeze(-1).broadcast_to([128, 2, E, R])
            nc.vector.tensor_tensor(out=wt24, in0=t24, in1=u24, op=ALU.mult)
            nc.vector.tensor_tensor(out=w4, in0=t14, in1=u14, op=ALU.mult)
            nc.vector.tensor_tensor(out=w_full, in0=w_full, in1=wt2, op=ALU.add)
            st["w_full"] = w_full

        def emit_m(st):
            w_full = st["w_full"]
            wrt_ps = ps_wrt.tile([ER, 2, 128], F32R, tag="wrt")
            for j in range(2):
                nc.tensor.transpose(
                    out=wrt_ps[:, j, :],
                    in_=w_full[:, j, :],
                    identity=ident_r,
                )
            wrt_sb = small_p.tile([ER, 2, 128], F32R, tag="wrt_sb")
            nc.any.tensor_copy(out=wrt_sb, in_=wrt_ps)
            lw_sb = lw_p.tile([ER, TT], F32R)
            nc.vector.scalar_tensor_tensor(
                out=lw_sb,
                in0=st["low_ps"],
                scalar=float(SCALING),
                in1=wrt_sb.rearrange("p j t -> p (j t)"),
                op0=ALU.mult,
                op1=ALU.mult,
            )
            st["lw_sb"] = lw_sb

        def emit_delta_pair(pair):
            """delta matmuls + PSUM->SBUF copies for both tiles of the pair
            into one [128, 4, H] output tile"""
            bo = bout_p.tile([128, 4, H], F32)
            pair["bo"] = bo
            for half in range(2):
                lw_r = pair["tiles"][half]["lw_sb"]
                for j in range(2):
                    q = 2 * half + j
                    dla = ps_dl.tile([128, 320], F32, tag="dla")
                    dlb = ps_dl.tile([128, 320], F32, tag="dlb")
                    nc.tensor.matmul(
                        out=dla, lhsT=lw_r[:, ts(j, 128)], rhs=bcat_sb[:, 0:320],
                        start=True, stop=True,
                    )
                    nc.tensor.matmul(
                        out=dlb, lhsT=lw_r[:, ts(j, 128)], rhs=bcat_sb[:, 320:H],
                        start=True, stop=True,
                    )
                    nc.any.tensor_copy(out=bo[:, q, 0:320], in_=dla)
                    nc.any.tensor_copy(out=bo[:, q, 320:H], in_=dlb)

        def emit_accum_pair(pair):
            """bo += base_output (one 1.28 MB SWDGE CCE-add), lagged so the
            Pool sequencer's wait on the copies is already satisfied"""
            nc.gpsimd.dma_start(
                out=pair["bo"],
                in_=base_d[pair["tok"] : pair["tok"] + PT, :].rearrange(
                    "(q p) h -> p q h", p=128
                ),
                accum_op=ALU.add,
            )

        def emit_store_pair(pair):
            """one 1.28 MB store on the ACT HWDGE ring, lagged so the ACT
            sequencer's wait on the accum-DMA is already satisfied"""
            nc.scalar.dma_start(
                out=out_d[pair["tok"] : pair["tok"] + PT, :].rearrange(
                    "(q p) h -> p q h", p=128
                ),
                in_=pair["bo"],
            )

        phist = []
        for p in range(passes):
            for pi in range(npairs):
                pair = emit_pair_load(pi)
                pair["tiles"] = []
                for half in range(2):
                    st = emit_front(pair, half)
                    emit_router(st)
                    emit_weights(st)
                    emit_m(st)
                    pair["tiles"].append(st)
                phist.append(pair)
                if len(phist) >= 2:
                    emit_delta_pair(phist[-2])
                if len(phist) >= 3:
                    emit_accum_pair(phist[-3])
                if len(phist) >= 4:
                    emit_store_pair(phist[-4])
        emit_delta_pair(phist[-1])
        if len(phist) >= 2:
            emit_accum_pair(phist[-2])
        emit_accum_pair(phist[-1])
        if len(phist) >= 3:
            emit_store_pair(phist[-3])
        if len(phist) >= 2:
            emit_store_pair(phist[-2])
        emit_store_pair(phist[-1])

    return nc


_CACHE = {}


def _get_nc(t_core=T_CORE, niter=1, timing_mode=False, passes=1):
    key = (t_core, niter, timing_mode, passes)
    if key not in _CACHE:
        nc = build_kernel(t_core, niter, timing_mode, passes)
        nc.finalize()
        _CACHE[key] = nc
    return _CACHE[key]


def kernel(x, base_output, W1, b1, W2, b2, A, Bm):
    x = np.ascontiguousarray(np.asarray(x), dtype=np.float32)
    base_output = np.ascontiguousarray(np.asarray(base_output), dtype=np.float32)
    W1 = np.ascontiguousarray(np.asarray(W1), dtype=np.float32)
    b1 = np.ascontiguousarray(np.asarray(b1), dtype=np.float32)
    W2 = np.ascontiguousarray(np.asarray(W2), dtype=np.float32)
    b2 = np.ascontiguousarray(np.asarray(b2), dtype=np.float32)
    A = np.ascontiguousarray(np.asarray(A), dtype=np.float32)
    Bm = np.ascontiguousarray(np.asarray(Bm), dtype=np.float32)

    B, S, _ = x.shape
    assert B * S == N_CORES * T_CORE
    xs = x.reshape(N_CORES, T_CORE, H)
    bs = base_output.reshape(N_CORES, T_CORE, H)

    nc = _get_nc()
    in_maps = [
        {
            "x": np.ascontiguousarray(xs[i]),
            "base": np.ascontiguousarray(bs[i]),
            "W1": W1, "b1": b1, "W2": W2, "b2": b2, "A": A, "Bm": Bm,
        }
        for i in range(N_CORES)
    ]
    res = run_bass_kernel_spmd(nc, in_maps, list(range(N_CORES))).results
    out = np.stack([res[i]["out"] for i in range(N_CORES)], axis=0)
    return out.reshape(B, S, H).astype(np.float32)


# revision 13
# speedup vs baseline: 1.3213x; 1.3213x over previous
"""MoLoRA Trainium2 Bass kernel — r12: TT=512 compute tiles.

Same DMA shape as r8 (1.28 MB per stream per tile, x on SP HWDGE, store on
ACT HWDGE, base-accum on gpsimd SWDGE; delta lag-1, accum lag-2, store
lag-3) but the compute tile is 512 tokens (4 partition-halves), cutting
DVE/ACT instruction count ~30% and PE instruction count ~25% at identical
element throughput.  PSUM fits 8 banks via per-h-chunk transpose staging
tiles ([128, 4, 128] = 1 bank) and the shared h/lg router tag.
"""

import numpy as np
from contextlib import ExitStack

import concourse.bass as bass
import concourse.tile as tile
from concourse import bacc
from concourse import mybir
from concourse.bass import ts
from concourse.masks import make_identity
from concourse.bass_utils import run_bass_kernel_spmd

F32 = mybir.dt.float32
F32R = mybir.dt.float32r
AF = mybir.ActivationFunctionType
ALU = mybir.AluOpType
AX = mybir.AxisListType

H = 640
E = 5
R = 8
ER = E * R
RH = 256
HC = H // 128
RC = RH // 128
SCALING = 16.0 / R
N_CORES = 8
T_CORE = 4096
TT = 512          # compute tile (4 halves of 128 tokens)
JT = TT // 128    # 4


def build_kernel(t_core=T_CORE, niter=1, timing_mode=False, passes=1):
    assert t_core % TT == 0
    ntiles = t_core // TT
    nc = bacc.Bacc()

    if timing_mode:
        x_d = nc.dram_tensor("x_int", [t_core, H], F32)[:, :]
        base_d = nc.dram_tensor("base_int", [t_core, H], F32)[:, :]
        out_d = nc.dram_tensor("out_int", [t_core, H], F32)[:, :]
        dummy_d = nc.declare_dram_parameter("dummy_out", [1, 4], F32, isOutput=True)
    else:
        x_d = nc.declare_dram_parameter("x", [t_core, H], F32, isOutput=False)
        base_d = nc.declare_dram_parameter("base", [t_core, H], F32, isOutput=False)
        out_d = nc.declare_dram_parameter("out", [t_core, H], F32, isOutput=True)
        dummy_d = None
    w1_d = nc.declare_dram_parameter("W1", [H, RH], F32, isOutput=False)
    b1_d = nc.declare_dram_parameter("b1", [RH], F32, isOutput=False)
    w2_d = nc.declare_dram_parameter("W2", [RH, E], F32, isOutput=False)
    b2_d = nc.declare_dram_parameter("b2", [E], F32, isOutput=False)
    a_d = nc.declare_dram_parameter("A", [E, H, R], F32, isOutput=False)
    bm_d = nc.declare_dram_parameter("Bm", [E, R, H], F32, isOutput=False)

    with ExitStack() as ctx:
        tc = ctx.enter_context(tile.TileContext(nc))
        const = ctx.enter_context(tc.tile_pool(name="const", bufs=1))
        xin_p = ctx.enter_context(tc.tile_pool(name="xin", bufs=3))
        bout_p = ctx.enter_context(tc.tile_pool(name="bout", bufs=5))
        xt_p = ctx.enter_context(tc.tile_pool(name="xt", bufs=2))
        ht_p = ctx.enter_context(tc.tile_pool(name="ht", bufs=2))
        small_p = ctx.enter_context(tc.tile_pool(name="small", bufs=4))
        lw_p = ctx.enter_context(tc.tile_pool(name="lw", bufs=3))
        ps_xt = ctx.enter_context(tc.tile_pool(name="ps_xt", bufs=1, space="PSUM"))
        ps_rt = ctx.enter_context(tc.tile_pool(name="ps_rt", bufs=1, space="PSUM"))
        ps_low = ctx.enter_context(tc.tile_pool(name="ps_low", bufs=2, space="PSUM"))
        ps_wrt = ctx.enter_context(tc.tile_pool(name="ps_wrt", bufs=1, space="PSUM"))
        ps_dl = ctx.enter_context(tc.tile_pool(name="ps_dl", bufs=1, space="PSUM"))

        ident = const.tile([128, 128], F32)
        make_identity(nc, ident)
        ident_r = const.tile([128, 128], F32R)
        nc.vector.tensor_copy(out=ident_r, in_=ident)

        w1_sb = const.tile([128, HC, RH], F32R)
        nc.gpsimd.dma_start(
            out=w1_sb, in_=w1_d.bitcast(F32R).rearrange("(c p) m -> p c m", p=128)
        )
        b1_sb = const.tile([128, RC], F32)
        nc.gpsimd.dma_start(out=b1_sb, in_=b1_d.rearrange("(c p) -> p c", p=128))
        w2_sb = const.tile([128, RC, E], F32)
        nc.gpsimd.dma_start(out=w2_sb, in_=w2_d.rearrange("(c p) e -> p c e", p=128))
        b2_sb = const.tile([1, E], F32)
        nc.gpsimd.dma_start(out=b2_sb, in_=b2_d[:].unsqueeze(0))
        ones_sb = const.tile([1, 128], F32)
        nc.vector.memset(ones_sb, 1.0)
        acat_sb = const.tile([128, HC, E, R], F32R)
        for e in range(E):
            for c in range(HC):
                nc.gpsimd.dma_start(
                    out=acat_sb[:, c, e, :],
                    in_=a_d.bitcast(F32R)[e, c * 128 : (c + 1) * 128, :],
                )
        bcat_sb = const.tile([ER, H], F32R)
        for e in range(E):
            nc.gpsimd.dma_start(
                out=bcat_sb[e * R : (e + 1) * R, :], in_=bm_d.bitcast(F32R)[e, :, :]
            )

        if dummy_d is not None:
            dnm = const.tile([1, 4], F32)
            nc.vector.memset(dnm, 1.0)
            nc.sync.dma_start(out=dummy_d[:, :], in_=dnm)

        loop_ctx = tc.For_i(0, niter, 1) if niter > 1 else None
        if loop_ctx is not None:
            ctx.enter_context(loop_ctx)

        def emit_front(i):
            """1.28 MB x load + xT transposes staged per h-chunk"""
            tok = i * TT
            x_nat = xin_p.tile([128, JT, H], F32R)
            nc.sync.dma_start(
                out=x_nat,
                in_=x_d.bitcast(F32R)[tok : tok + TT, :].rearrange(
                    "(q p) h -> p q h", p=128
                ),
            )
            xt_sb = xt_p.tile([128, HC, TT], F32R)
            for c in range(HC):
                xtc = ps_xt.tile([128, JT, 128], F32R, tag="xtp")
                for q in range(JT):
                    nc.tensor.transpose(
                        out=xtc[:, q, :],
                        in_=x_nat[:, q, ts(c, 128)],
                        identity=ident_r,
                    )
                nc.any.tensor_copy(
                    out=xt_sb[:, c, :].rearrange("p (q t) -> p q t", q=JT), in_=xtc
                )
            return {"xt_sb": xt_sb, "tok": tok}

        def emit_router(st):
            xt_r = st["xt_sb"]
            h_ps = ps_rt.tile([128, RC, TT], F32, tag="rt")
            for c2 in range(RC):
                for c in range(HC):
                    nc.tensor.matmul(
                        out=h_ps[:, c2, :],
                        lhsT=w1_sb[:, c, ts(c2, 128)],
                        rhs=xt_r[:, c, :],
                        start=(c == 0),
                        stop=(c == HC - 1),
                    )
            ht_sb = ht_p.tile([128, RC, TT], F32)
            sg_sb = ht_p.tile([128, RC, TT], F32, tag="sg")
            for c2 in range(RC):
                nc.scalar.activation(
                    out=sg_sb[:, c2, :],
                    in_=h_ps[:, c2, :],
                    func=AF.Sigmoid,
                    bias=b1_sb[:, c2 : c2 + 1],
                )
                nc.vector.scalar_tensor_tensor(
                    out=ht_sb[:, c2, :],
                    in0=h_ps[:, c2, :],
                    scalar=b1_sb[:, c2 : c2 + 1],
                    in1=sg_sb[:, c2, :],
                    op0=ALU.add,
                    op1=ALU.mult,
                )

            low_ps = ps_low.tile([ER, TT], F32, tag="low")
            for c in range(HC):
                nc.tensor.matmul(
                    out=low_ps,
                    lhsT=acat_sb[:, c, :, :],
                    rhs=xt_r[:, c, :],
                    start=(c == 0),
                    stop=(c == HC - 1),
                )
            st["low_ps"] = low_ps

            # token-major logits for the 4 q-halves, packed 2-per-rt-half so
            # each [128, E] matmul output stays inside one PSUM bank
            lg_full = ps_rt.tile([128, RC, TT], F32, tag="rt")
            lg4 = lg_full[:, :, 0 : 2 * E].rearrange("p a (b e) -> p a b e", e=E)
            for q in range(JT):
                for c2 in range(RC):
                    nc.tensor.matmul(
                        out=lg4[:, q // 2, q % 2, :],
                        lhsT=ht_sb[:, c2, ts(q, 128)],
                        rhs=w2_sb[:, c2, :],
                        start=(c2 == 0),
                        stop=False,
                    )
                nc.tensor.matmul(
                    out=lg4[:, q // 2, q % 2, :],
                    lhsT=ones_sb,
                    rhs=b2_sb,
                    start=False,
                    stop=True,
                )

            # top-2 fused over all 4 q-halves via [128, 2, 2, E] views
            m1 = small_p.tile([128, JT], F32, tag="m1")
            m1_4 = m1.rearrange("p (a b) -> p a b", b=2)
            nc.vector.tensor_reduce(out=m1_4, in_=lg4, axis=AX.X, op=ALU.max)
            top1 = small_p.tile([128, JT, E], F32, tag="top1")
            top1_4 = top1.rearrange("p (a b) e -> p a b e", b=2)
            nc.vector.tensor_tensor(
                out=top1_4,
                in0=lg4,
                in1=m1_4.unsqueeze(-1).broadcast_to([128, 2, 2, E]),
                op=ALU.is_equal,
            )
            masked = small_p.tile([128, JT, E], F32, tag="masked")
            masked_4 = masked.rearrange("p (a b) e -> p a b e", b=2)
            nc.vector.scalar_tensor_tensor(
                out=masked_4, in0=top1_4, scalar=-1e30, in1=lg4,
                op0=ALU.mult, op1=ALU.add,
            )
            m2 = small_p.tile([128, JT], F32, tag="m2")
            nc.vector.tensor_reduce(out=m2, in_=masked, axis=AX.X, op=ALU.max)
            dlg = small_p.tile([128, JT], F32, tag="dlg")
            nc.vector.tensor_tensor(out=dlg, in0=m2, in1=m1, op=ALU.subtract)
            st["top1"], st["masked"], st["m2"], st["dlg"] = top1, masked, m2, dlg

        def emit_weights(st):
            top1, masked, m2, dlg = st["top1"], st["masked"], st["m2"], st["dlg"]
            u2 = small_p.tile([128, JT], F32, tag="u2")
            nc.scalar.activation(out=u2, in_=dlg, func=AF.Sigmoid)
            u1 = small_p.tile([128, JT], F32, tag="u1")
            nc.vector.tensor_scalar(
                out=u1, in0=u2, scalar1=-1.0, scalar2=1.0,
                op0=ALU.mult, op1=ALU.add,
            )
            top2 = small_p.tile([128, JT, E], F32, tag="top2")
            nc.vector.tensor_tensor(
                out=top2,
                in0=masked,
                in1=m2.unsqueeze(-1).broadcast_to([128, JT, E]),
                op=ALU.is_equal,
            )
            w_full = small_p.tile([128, JT, ER], F32R)
            wt2 = small_p.tile([128, JT, ER], F32, tag="wt2")
            w4 = w_full.rearrange("p j (e r) -> p j e r", r=R)
            wt24 = wt2.rearrange("p j (e r) -> p j e r", r=R)
            t14 = top1.unsqueeze(-1).broadcast_to([128, JT, E, R])
            t24 = top2.unsqueeze(-1).broadcast_to([128, JT, E, R])
            u14 = u1.unsqueeze(-1).unsqueeze(-1).broadcast_to([128, JT, E, R])
            u24 = u2.unsqueeze(-1).unsqueeze(-1).broadcast_to([128, JT, E, R])
            nc.vector.tensor_tensor(out=wt24, in0=t24, in1=u24, op=ALU.mult)
            nc.vector.tensor_tensor(out=w4, in0=t14, in1=u14, op=ALU.mult)
            nc.vector.tensor_tensor(out=w_full, in0=w_full, in1=wt2, op=ALU.add)
            st["w_full"] = w_full

        def emit_m(st):
            w_full = st["w_full"]
            wrt_ps = ps_wrt.tile([ER, JT, 128], F32R, tag="wrt")
            for q in range(JT):
                nc.tensor.transpose(
                    out=wrt_ps[:, q, :],
                    in_=w_full[:, q, :],
                    identity=ident_r,
                )
            wrt_sb = small_p.tile([ER, JT, 128], F32R, tag="wrt_sb")
            nc.any.tensor_copy(out=wrt_sb, in_=wrt_ps)
            lw_sb = lw_p.tile([ER, TT], F32R)
            nc.vector.scalar_tensor_tensor(
                out=lw_sb,
                in0=st["low_ps"],
                scalar=float(SCALING),
                in1=wrt_sb.rearrange("p j t -> p (j t)"),
                op0=ALU.mult,
                op1=ALU.mult,
            )
            st["lw_sb"] = lw_sb

        def emit_delta(st):
            lw_r = st["lw_sb"]
            bo = bout_p.tile([128, JT, H], F32)
            st["bo"] = bo
            for q in range(JT):
                dla = ps_dl.tile([128, 320], F32, tag="dla")
                dlb = ps_dl.tile([128, 320], F32, tag="dlb")
                nc.tensor.matmul(
                    out=dla, lhsT=lw_r[:, ts(q, 128)], rhs=bcat_sb[:, 0:320],
                    start=True, stop=True,
                )
                nc.tensor.matmul(
                    out=dlb, lhsT=lw_r[:, ts(q, 128)], rhs=bcat_sb[:, 320:H],
                    start=True, stop=True,
                )
                nc.any.tensor_copy(out=bo[:, q, 0:320], in_=dla)
                nc.any.tensor_copy(out=bo[:, q, 320:H], in_=dlb)

        def emit_accum(st):
            nc.gpsimd.dma_start(
                out=st["bo"],
                in_=base_d[st["tok"] : st["tok"] + TT, :].rearrange(
                    "(q p) h -> p q h", p=128
                ),
                accum_op=ALU.add,
            )

        def emit_store(st):
            nc.scalar.dma_start(
                out=out_d[st["tok"] : st["tok"] + TT, :].rearrange(
                    "(q p) h -> p q h", p=128
                ),
                in_=st["bo"],
            )

        hist = []
        for p in range(passes):
            for i in range(ntiles):
                st = emit_front(i)
                emit_router(st)
                emit_weights(st)
                emit_m(st)
                hist.append(st)
                if len(hist) >= 2:
                    emit_delta(hist[-2])
                if len(hist) >= 3:
                    emit_accum(hist[-3])
                if len(hist) >= 4:
                    emit_store(hist[-4])
        emit_delta(hist[-1])
        if len(hist) >= 2:
            emit_accum(hist[-2])
        emit_accum(hist[-1])
        if len(hist) >= 3:
            emit_store(hist[-3])
        if len(hist) >= 2:
            emit_store(hist[-2])
        emit_store(hist[-1])

    return nc


_CACHE = {}


def _get_nc(t_core=T_CORE, niter=1, timing_mode=False, passes=1):
    key = (t_core, niter, timing_mode, passes)
    if key not in _CACHE:
        nc = build_kernel(t_core, niter, timing_mode, passes)
        nc.finalize()
        _CACHE[key] = nc
    return _CACHE[key]


def kernel(x, base_output, W1, b1, W2, b2, A, Bm):
    x = np.ascontiguousarray(np.asarray(x), dtype=np.float32)
    base_output = np.ascontiguousarray(np.asarray(base_output), dtype=np.float32)
    W1 = np.ascontiguousarray(np.asarray(W1), dtype=np.float32)
    b1 = np.ascontiguousarray(np.asarray(b1), dtype=np.float32)
    W2 = np.ascontiguousarray(np.asarray(W2), dtype=np.float32)
    b2 = np.ascontiguousarray(np.asarray(b2), dtype=np.float32)
    A = np.ascontiguousarray(np.asarray(A), dtype=np.float32)
    Bm = np.ascontiguousarray(np.asarray(Bm), dtype=np.float32)

    B, S, _ = x.shape
    assert B * S == N_CORES * T_CORE
    xs = x.reshape(N_CORES, T_CORE, H)
    bs = base_output.reshape(N_CORES, T_CORE, H)

    nc = _get_nc()
    in_maps = [
        {
            "x": np.ascontiguousarray(xs[i]),
            "base": np.ascontiguousarray(bs[i]),
            "W1": W1, "b1": b1, "W2": W2, "b2": b2, "A": A, "Bm": Bm,
        }
        for i in range(N_CORES)
    ]
    res = run_bass_kernel_spmd(nc, in_maps, list(range(N_CORES))).results
    out = np.stack([res[i]["out"] for i in range(N_CORES)], axis=0)
    return out.reshape(B, S, H).astype(np.float32)
